# revision 2
# baseline (speedup 1.0000x reference)
"""Trainium2 Bass kernel for nn_AdaptiveAdjacencyMatrix.

Reference math:
    s[b, i]        = sum_d h[b, i, d] * w[d]
    scores[b,i,j]  = s[b,i] + s[b,j] + bias
    A              = softmax(scores, axis=1)   # over i

Because the softmax is over axis=1 (i), the `s[b,j] + bias` term is constant
along the reduced axis and cancels exactly:
    A[b, i, j] = exp(s[b,i]) / sum_i' exp(s[b,i'])   (independent of j and bias)

So the output is a column-broadcast of softmax(s[b]) — the kernel is purely
memory-bound: write 4*4096*4096*4 = 268 MB of output at HBM speed.

Sharding: 8 cores = (batch b, row-half rh). Each core receives the full
h[b] (rows reordered so its own 2048 rows come first), computes softmax(s)
locally (the softmax sum needs all 4096 rows anyway; row order is
irrelevant to the sum), and writes a [2048, 4096] output shard. No
collectives needed.

Layouts: h is DMA'd with fully-contiguous per-partition descriptors
(partition p holds rows 16p..16p+15 of its half), which makes the on-chip
softmax come out in a (q, r) layout where device output row 128r + q holds
the value for local input row 16q + r. The host unshard undoes that with a
cheap reshape/transpose.
"""

import ml_dtypes
import numpy as np

B, N, D = 4, 4096, 256
NCORES = 8
HALF = N // 2          # 2048 rows written per core
P = 128                # SBUF partitions
RPP = HALF // P        # 16 rows per partition (per half)
DOT_CHUNK = 4          # rows-per-partition per h-load/dot chunk (512 KB)
SUP = 2                # groups per output supertile / DMA (4 MB)

_CACHE = {}


def _build():
    import concourse.mybir as mybir
    import concourse.tile as tile
    from concourse import bacc

    f32 = mybir.dt.float32
    Copy = mybir.ActivationFunctionType.Copy
    nc = bacc.Bacc("TRN2", target_bir_lowering=False, debug=False)

    bf16 = mybir.dt.bfloat16
    h_ext = nc.declare_dram_parameter("h", [N, D], bf16, isOutput=False)
    # w arrives pre-broadcast to 128 partitions (host-side tile of the 1 KB
    # vector) so the dot-product chain only waits on one small DMA.
    w_ext = nc.declare_dram_parameter("wb", [P, D], bf16, isOutput=False)
    out_ext = nc.declare_dram_parameter("out", [HALF, N], f32, isOutput=True)

    # contiguous flat views: partition p holds rows 16p..16p+15 of each half
    h_mine = h_ext[0:HALF, :].rearrange("(p r) d -> p r d", p=P)
    h_oth = h_ext[HALF:N, :].rearrange("(p r) d -> p r d", p=P)
    # [128, r, j] view of out: device out row = 128r + q (host un-permutes)
    out_r = out_ext[:, :].rearrange("(r q) j -> q r j", q=P)

    with tile.TileContext(nc) as tc:
        with (
            tc.tile_pool(name="const", bufs=1) as cpool,
            tc.tile_pool(name="hload", bufs=4) as hpool,
            tc.tile_pool(name="prod", bufs=4) as ppool,
            tc.tile_pool(name="small", bufs=1) as spool,
            tc.tile_pool(name="obuf", bufs=3) as opool,
            tc.tile_pool(name="psum", bufs=1, space="PSUM") as psum_pool,
        ):
            # all-ones [128,128] for the PE cross-partition-sum trick
            ones_k = cpool.tile([P, P], f32)
            nc.vector.memset(ones_k[:, :], 1.0)

            # --- w, pre-broadcast on host; first on the sync ring ---
            w_bc = cpool.tile([P, D], bf16)
            nc.sync.dma_start(out=w_bc[:, :], in_=w_ext[:, :])
            # materialized repeat (real strides, so DVE 2x bf16 mode applies)
            w_rep = cpool.tile([P, DOT_CHUNK, D], bf16)
            nc.vector.tensor_copy(
                w_rep[:, :, :],
                w_bc[:, :].unsqueeze(1).broadcast_to([P, DOT_CHUNK, D]),
            )

            # --- s = h @ w for both halves, [128, 16] each.
            # All h DMAs go on the sync ring only (FIFO: first chunk lands
            # fast; the scalar ring's DMAs would queue behind ACT compute).
            # DVE does all multiplies; reductions split ACT/DVE to balance
            # (ACT accum-reduce costs ~0.69us per row vs DVE 1.22us per
            # 4-row chunk). ---
            s_mine = spool.tile([P, RPP], f32)
            s_oth = spool.tile([P, RPP], f32)
            e_mine = spool.tile([P, RPP], f32)
            rs_m = spool.tile([P, 1], f32)
            rs_o = spool.tile([P, 1], f32)
            tot_psum = psum_pool.tile([P, 1], f32)
            jnk = spool.tile([P, D], f32)
            # reduction assignment per chunk: ACT accum-reduce for these,
            # DVE tensor_reduce for the rest (balances the two engines)
            act_chunks = {0, 1, 3, 5}
            n_chunks = RPP // DOT_CHUNK  # 4 per half
            for half in range(2):
                h_src = h_mine if half == 0 else h_oth
                s_dst = s_mine if half == 0 else s_oth
                for c in range(n_chunks):
                    ci = half * n_chunks + c
                    hch = hpool.tile([P, DOT_CHUNK, D], bf16)
                    nc.sync.dma_start(
                        out=hch[:, :, :],
                        in_=h_src[:, c * DOT_CHUNK : (c + 1) * DOT_CHUNK, :],
                    )
                    prod = ppool.tile([P, DOT_CHUNK, D], bf16)
                    nc.vector.tensor_tensor(
                        out=prod[:, :, :],
                        in0=hch[:, :, :],
                        in1=w_rep[:, :, :],
                        op=mybir.AluOpType.mult,
                    )
                    if ci in act_chunks:
                        for g in range(DOT_CHUNK):
                            gi = c * DOT_CHUNK + g
                            nc.scalar.activation(
                                out=jnk[:, :],
                                in_=prod[:, g, :],
                                func=Copy,
                                accum_out=s_dst[:, gi : gi + 1],
                            )
                    else:
                        nc.vector.tensor_reduce(
                            out=s_dst[:, c * DOT_CHUNK : (c + 1) * DOT_CHUNK],
                            in_=prod[:, :, :],
                            axis=mybir.AxisListType.X,
                            op=mybir.AluOpType.add,
                        )
                if half == 0:
                    # mine half done: exp + its partition-sum can overlap the
                    # other half's dot products
                    nc.scalar.activation(
                        out=e_mine[:, :],
                        in_=s_mine[:, :],
                        func=mybir.ActivationFunctionType.Exp,
                        accum_out=rs_m[:, 0:1],
                    )
                    # route rs through DVE so the PE matmul needs only one
                    # wait (its LdWeights slot fits a single semaphore)
                    rs_m2 = spool.tile([P, 1], f32)
                    nc.vector.tensor_copy(rs_m2[:, 0:1], rs_m[:, 0:1])
                    nc.tensor.matmul(
                        tot_psum[:, 0:1],
                        ones_k[:, 0:P],
                        rs_m2[:, 0:1],
                        start=True,
                        stop=False,
                    )

            # --- finish the softmax sum: exp(other), accumulate its
            # partition-sum into the same PSUM bank, reciprocal, scale ---
            e_oth = spool.tile([P, RPP], f32)
            nc.scalar.activation(
                out=e_oth[:, :],
                in_=s_oth[:, :],
                func=mybir.ActivationFunctionType.Exp,
                accum_out=rs_o[:, 0:1],
            )
            nc.tensor.matmul(
                tot_psum[:, 0:1], ones_k[:, 0:P], rs_o[:, 0:1], start=False, stop=True
            )
            inv = spool.tile([P, 1], f32)
            nc.vector.reciprocal(inv[:, 0:1], tot_psum[:, 0:1])

            # --- broadcast p along columns (stride-0 reads) and stream out.
            # First supertile is a single group so the DMA stream (the
            # rate-limiting resource) starts one broadcast-op earlier. ---
            tiles = [1] + [SUP] * ((RPP - 1) // SUP) + (
                [RPP - 1 - SUP * ((RPP - 1) // SUP)]
                if (RPP - 1) % SUP
                else []
            )
            gi = 0
            for t, width in enumerate(tiles):
                ot = opool.tile([P, SUP * N], f32, tag="ot")
                for g in range(width):
                    # p = e * (1/S) folded into the broadcast multiply
                    col_b = e_mine[:, gi + g : gi + g + 1].broadcast_to([P, N])
                    dst = ot[:, g * N : (g + 1) * N]
                    nc.vector.tensor_scalar_mul(dst, col_b, inv[:, 0:1])
                # alternate the two HWDGE rings for more DMA queue depth
                out_dma_eng = nc.sync if t % 2 == 0 else nc.scalar
                out_dma_eng.dma_start(
                    out=out_r[:, gi : gi + width, :],
                    in_=ot[:, 0 : width * N].rearrange("q (r j) -> q r j", r=width),
                )
                gi += width
    nc.compile()
    return nc


def _get_nc():
    if "nc" not in _CACHE:
        _CACHE["nc"] = _build()
    return _CACHE["nc"]


def _ensure_axon_hooks():
    """bass_utils' trace path imports antenv.axon_hooks, which some images
    lack; provide a stub so tracing degrades instead of crashing. If the
    boot package + libaxon_pjrt.so are present, register the real
    ctypes-based NTFF profile hook so traced runs report exec_time_ns."""
    import sys
    import types

    try:
        import antenv.axon_hooks as m
    except ImportError:
        try:
            import antenv
        except ImportError:
            antenv = types.ModuleType("antenv")
            sys.modules["antenv"] = antenv
        m = types.ModuleType("antenv.axon_hooks")
        m._hook = None
        m.set_axon_ntff_profile_hook = lambda h: setattr(m, "_hook", h)
        m.get_axon_ntff_profile_hook = lambda: m._hook
        sys.modules["antenv.axon_hooks"] = m
    if m.get_axon_ntff_profile_hook() is None:
        try:
            import os

            from trn_agent_boot.trn_boot import _ntff_profile_via_ctypes

            so_path = "/opt/axon/libaxon_pjrt.so"
            if os.path.exists(so_path):
                hook = _ntff_profile_via_ctypes(so_path)
                if hook is not None:
                    m.set_axon_ntff_profile_hook(hook)
        except Exception:
            pass


def run_on_device(h, w, trace=False):
    """Run the SPMD kernel; returns the BassKernelResults."""
    from concourse.bass_utils import run_bass_kernel_spmd

    _ensure_axon_hooks()

    in_maps = []
    for c in range(NCORES):
        b_idx, rh = divmod(c, 2)
        hb = h[b_idx]
        if rh:
            hb = np.concatenate([hb[HALF:], hb[:HALF]], axis=0)
        in_maps.append(
            {
                "h": np.ascontiguousarray(hb.astype(ml_dtypes.bfloat16)),
                "wb": np.ascontiguousarray(
                    np.broadcast_to(w.astype(ml_dtypes.bfloat16), (P, D))
                ),
            }
        )
    res = run_bass_kernel_spmd(
        _get_nc(), in_maps, core_ids=list(range(NCORES)), trace=trace
    )
    return res


def kernel(h, w, b):
    h = np.asarray(h, dtype=np.float32)
    w = np.asarray(w, dtype=np.float32)
    res = run_on_device(h, w)
    A = np.empty((B, N, N), dtype=np.float32)
    for c in range(NCORES):
        b_idx, rh = divmod(c, 2)
        out_c = res.results[c]["out"]
        # device row 128r + q holds the value for local input row 16q + r:
        # undo with reshape/transpose
        unperm = (
            out_c.reshape(RPP, P, N).transpose(1, 0, 2).reshape(HALF, N)
        )
        A[b_idx, rh * HALF : (rh + 1) * HALF, :] = unperm
    return A



# revision 3
# speedup vs baseline: 1.3002x; 1.3002x over previous
"""Trainium2 Bass kernel for nn_AdaptiveAdjacencyMatrix.

Reference math:
    s[b, i]        = sum_d h[b, i, d] * w[d]
    scores[b,i,j]  = s[b,i] + s[b,j] + bias
    A              = softmax(scores, axis=1)   # over i

Because the softmax is over axis=1 (i), the `s[b,j] + bias` term is constant
along the reduced axis and cancels exactly:
    A[b, i, j] = exp(s[b,i]) / sum_i' exp(s[b,i'])   (independent of j and bias)

So the output is a column-broadcast of softmax(s[b]) — the kernel is purely
memory-bound. The output is written in bf16 (the host upcasts to f32), which
halves HBM write traffic vs f32; quantization error ~2^-9 is far inside the
accuracy budget.

Sharding: 8 cores = (batch b, row-half rh). Each core receives the full
h[b] (rows reordered so its own 2048 rows come LAST; the other half comes
first so its dot products are off the critical path), computes the full
softmax sum locally (needs all 4096 rows; row order is irrelevant to the
sum), and writes a [2048, 4096] bf16 output shard. No collectives.

Layouts: h is DMA'd with contiguous per-partition descriptors (partition p
holds rows 16p..16p+15 of a half, 4 KB runs per chunk). The output uses the
matching (q r) layout — device row q*16 + r holds the value for input row
16q + r — so the returned shard is already in natural row order (no host
permute) and each partition writes contiguous 8 KB+ HBM runs.
"""

import ml_dtypes
import numpy as np

B, N, D = 4, 4096, 256
NCORES = 8
HALF = N // 2          # 2048 rows written per core
P = 128                # SBUF partitions
RPP = HALF // P        # 16 rows per partition (per half)
CH = 8                 # rows-per-partition per h-load/dot chunk (512 KB)
NG = RPP               # 16 output groups of [P, N] each (1 MB bf16)

_CACHE = {}


def _build():
    import concourse.mybir as mybir
    import concourse.tile as tile
    from concourse import bacc

    f32 = mybir.dt.float32
    bf16 = mybir.dt.bfloat16
    Copy = mybir.ActivationFunctionType.Copy
    Exp = mybir.ActivationFunctionType.Exp
    AX = mybir.AxisListType.X
    ADD = mybir.AluOpType.add
    MUL = mybir.AluOpType.mult
    nc = bacc.Bacc("TRN2", target_bir_lowering=False, debug=False)

    h_ext = nc.declare_dram_parameter("h", [N, D], bf16, isOutput=False)
    # w arrives pre-tiled to [P, CH, D] on host so the dot-product multiplies
    # read real-strided bf16 (keeps DVE 2x mode) with no on-chip repeat op.
    w_ext = nc.declare_dram_parameter("wb", [P, CH, D], bf16, isOutput=False)
    out_ext = nc.declare_dram_parameter("out", [HALF, N], bf16, isOutput=True)

    # contiguous flat views: partition p holds rows 16p..16p+15 of each half
    h_oth = h_ext[0:HALF, :].rearrange("(p r) d -> p r d", p=P)
    h_mine = h_ext[HALF:N, :].rearrange("(p r) d -> p r d", p=P)
    # (q r) view of out: device row q*16 + r <-> e[q, r] (input row 16q + r),
    # so the shard comes back in natural order and partition q's writes are
    # contiguous in HBM.
    out_q = out_ext[:, :].rearrange("(q r) j -> q r j", r=RPP)

    with tile.TileContext(nc) as tc:
        with (
            tc.tile_pool(name="const", bufs=1) as cpool,
            tc.tile_pool(name="hload", bufs=4) as hpool,
            tc.tile_pool(name="prod", bufs=4) as ppool,
            tc.tile_pool(name="small", bufs=1) as spool,
            tc.tile_pool(name="obuf", bufs=4) as opool,
            tc.tile_pool(name="psum", bufs=1, space="PSUM") as psum_pool,
        ):
            # all-ones [128,128] for the PE cross-partition-sum trick
            ones_k = cpool.tile([P, P], f32)
            nc.vector.memset(ones_k[:, :], 1.0)

            # --- w (pre-tiled on host); first on the sync ring ---
            w_rep = cpool.tile([P, CH, D], bf16)
            nc.sync.dma_start(out=w_rep[:, :, :], in_=w_ext[:, :, :])

            # --- s = h @ w, other half first (its rows only feed the softmax
            # sum), own half second. All h DMAs on the sync ring, queued
            # up-front (bufs cover all chunks). Per 8-row chunk: DVE does the
            # elementwise multiply; rows 0-1 reduce on ACT (accum-reduce),
            # rows 2-7 in one batched DVE tensor_reduce — balances engines. ---
            s_oth = spool.tile([P, RPP], f32)
            s_mine = spool.tile([P, RPP], f32)
            e_oth = spool.tile([P, RPP], f32)
            e_mine = spool.tile([P, RPP], f32)
            rs = spool.tile([P, 2], f32)
            jnk = spool.tile([P, D], f32)
            ACT_ROWS = 2  # leading rows per chunk reduced on ACT

            chunks = [
                (h_oth, 0, s_oth), (h_oth, CH, s_oth),
                (h_mine, 0, s_mine), (h_mine, CH, s_mine),
            ]
            for h_src, r0, s_dst in chunks:
                hch = hpool.tile([P, CH, D], bf16)
                nc.sync.dma_start(out=hch[:, :, :], in_=h_src[:, r0 : r0 + CH, :])
                prod = ppool.tile([P, CH, D], bf16)
                nc.vector.tensor_tensor(
                    out=prod[:, :, :], in0=hch[:, :, :], in1=w_rep[:, :, :], op=MUL
                )
                for g in range(ACT_ROWS):
                    nc.scalar.activation(
                        out=jnk[:, :],
                        in_=prod[:, g, :],
                        func=Copy,
                        accum_out=s_dst[:, r0 + g : r0 + g + 1],
                    )
                nc.vector.tensor_reduce(
                    out=s_dst[:, r0 + ACT_ROWS : r0 + CH],
                    in_=prod[:, ACT_ROWS:CH, :],
                    axis=AX,
                    op=ADD,
                )
                if r0 == CH:  # half complete -> exp + row-sum accumulate
                    hi = 0 if s_dst is s_oth else 1
                    e_dst = e_oth if s_dst is s_oth else e_mine
                    nc.scalar.activation(
                        out=e_dst[:, :],
                        in_=s_dst[:, :],
                        func=Exp,
                        accum_out=rs[:, hi : hi + 1],
                    )

            # --- total sum via PE ones-matmul (sums partitions, broadcasts
            # the result to every partition), then 1/S and p = e/S ---
            rs2 = spool.tile([P, 2], f32)
            nc.vector.tensor_copy(rs2[:, :], rs[:, :])  # single producer for PE
            tot_psum = psum_pool.tile([P, 2], f32)
            nc.tensor.matmul(
                tot_psum[:, 0:2], ones_k[:, 0:P], rs2[:, 0:2], start=True, stop=True
            )
            tot = spool.tile([P, 1], f32)
            nc.vector.tensor_reduce(out=tot[:, 0:1], in_=tot_psum[:, 0:2], axis=AX, op=ADD)
            inv = spool.tile([P, 1], f32)
            nc.vector.reciprocal(inv[:, 0:1], tot[:, 0:1])
            p_mine = spool.tile([P, RPP], f32)
            nc.vector.tensor_scalar_mul(p_mine[:, :], e_mine[:, :], inv[:, 0:1])

            # --- broadcast p along columns (stride-0 reads) into bf16 tiles
            # and stream out on both HWDGE rings. Schedule: two half-group
            # DMAs first (earliest first byte), one full group each on ACT
            # and GpSimd (frees DVE), then 2-group supertiles on DVE. ---
            def bcast(eng, dst, src_col):
                if eng == "act":
                    nc.scalar.activation(out=dst, in_=src_col, func=Copy)
                elif eng == "gps":
                    nc.gpsimd.tensor_copy(dst, src_col)
                else:
                    nc.vector.tensor_copy(dst, src_col)

            # (n_groups, engine) per supertile; groups assigned sequentially
            sched = [(1, "dve"), (1, "act"), (1, "gps")] + [(2, "dve")] * 6 + [
                (1, "dve")
            ]
            gi = 0
            for t, (width, eng) in enumerate(sched):
                ot = opool.tile([P, 2 * N], bf16, tag="ot")
                if t == 0:
                    # first group split into two half-row DMAs for fast start
                    for hj in range(2):
                        j0 = hj * (N // 2)
                        bcast(
                            eng,
                            ot[:, j0 : j0 + N // 2],
                            p_mine[:, 0:1].broadcast_to([P, N // 2]),
                        )
                        dma_eng = nc.sync if hj == 0 else nc.scalar
                        dma_eng.dma_start(
                            out=out_q[:, 0:1, j0 : j0 + N // 2],
                            in_=ot[:, j0 : j0 + N // 2].rearrange(
                                "q (r j) -> q r j", r=1
                            ),
                        )
                    gi += 1
                    continue
                if width == 1:
                    bcast(eng, ot[:, 0:N], p_mine[:, gi : gi + 1].broadcast_to([P, N]))
                else:
                    bcast(
                        eng,
                        ot[:, 0 : 2 * N].rearrange("q (r j) -> q r j", r=2),
                        p_mine[:, gi : gi + 2].unsqueeze(2).broadcast_to([P, 2, N]),
                    )
                dma_eng = nc.sync if t % 2 == 0 else nc.scalar
                dma_eng.dma_start(
                    out=out_q[:, gi : gi + width, :],
                    in_=ot[:, 0 : width * N].rearrange("q (r j) -> q r j", r=width),
                )
                gi += width
            assert gi == NG
    nc.compile()
    return nc


def _get_nc():
    if "nc" not in _CACHE:
        _CACHE["nc"] = _build()
    return _CACHE["nc"]


def _ensure_axon_hooks():
    """bass_utils' trace path imports antenv.axon_hooks, which some images
    lack; provide a stub so tracing degrades instead of crashing. If the
    boot package + libaxon_pjrt.so are present, register the real
    ctypes-based NTFF profile hook so traced runs report exec_time_ns."""
    import sys
    import types

    try:
        import antenv.axon_hooks as m
    except ImportError:
        try:
            import antenv
        except ImportError:
            antenv = types.ModuleType("antenv")
            sys.modules["antenv"] = antenv
        m = types.ModuleType("antenv.axon_hooks")
        m._hook = None
        m.set_axon_ntff_profile_hook = lambda h: setattr(m, "_hook", h)
        m.get_axon_ntff_profile_hook = lambda: m._hook
        sys.modules["antenv.axon_hooks"] = m
    if m.get_axon_ntff_profile_hook() is None:
        try:
            import os

            from trn_agent_boot.trn_boot import _ntff_profile_via_ctypes

            so_path = "/opt/axon/libaxon_pjrt.so"
            if os.path.exists(so_path):
                hook = _ntff_profile_via_ctypes(so_path)
                if hook is not None:
                    m.set_axon_ntff_profile_hook(hook)
        except Exception:
            pass


def run_on_device(h, w, trace=False):
    """Run the SPMD kernel; returns the BassKernelResults."""
    from concourse.bass_utils import run_bass_kernel_spmd

    _ensure_axon_hooks()

    wb = np.ascontiguousarray(
        np.broadcast_to(w.astype(ml_dtypes.bfloat16), (P, CH, D))
    )
    in_maps = []
    for c in range(NCORES):
        b_idx, rh = divmod(c, 2)
        hb = h[b_idx]
        # other half first (off the critical path), own half second
        if rh:
            hb_dev = hb
        else:
            hb_dev = np.concatenate([hb[HALF:], hb[:HALF]], axis=0)
        in_maps.append(
            {
                "h": np.ascontiguousarray(hb_dev.astype(ml_dtypes.bfloat16)),
                "wb": wb,
            }
        )
    res = run_bass_kernel_spmd(
        _get_nc(), in_maps, core_ids=list(range(NCORES)), trace=trace
    )
    return res


def kernel(h, w, b):
    h = np.asarray(h, dtype=np.float32)
    w = np.asarray(w, dtype=np.float32)
    res = run_on_device(h, w)
    A = np.empty((B, N, N), dtype=np.float32)
    for c in range(NCORES):
        b_idx, rh = divmod(c, 2)
        A[b_idx, rh * HALF : (rh + 1) * HALF, :] = res.results[c]["out"].astype(
            np.float32
        )
    return A


# revision 7
# speedup vs baseline: 1.4050x; 1.0806x over previous
"""Trainium2 Bass kernel for nn_AdaptiveAdjacencyMatrix.

Reference math:
    s[b, i]        = sum_d h[b, i, d] * w[d]
    scores[b,i,j]  = s[b,i] + s[b,j] + bias
    A              = softmax(scores, axis=1)   # over i

Because the softmax is over axis=1 (i), the `s[b,j] + bias` term is constant
along the reduced axis and cancels exactly:
    A[b, i, j] = exp(s[b,i]) / sum_i' exp(s[b,i'])   (independent of j and bias)

So the output is a column-broadcast of softmax(s[b]) — the kernel is purely
memory-bound. The output is written in bf16 (the host upcasts to f32), which
halves HBM write traffic vs f32; quantization error ~2^-9 is far inside the
accuracy budget.

Sharding: 8 cores = (batch b, row-half rh). Each core receives the full
h[b] (rows reordered so its own 2048 rows come LAST; the other half comes
first so its dot products are off the critical path), computes the full
softmax sum locally (needs all 4096 rows; row order is irrelevant to the
sum), and writes a [2048, 4096] bf16 output shard. No collectives.

Layouts: h is DMA'd with contiguous per-partition descriptors (partition p
holds rows 16p..16p+15 of a half, 4 KB runs per chunk). The output uses the
matching (q r) layout — device row q*16 + r holds the value for input row
16q + r — so the returned shard is already in natural row order (no host
permute) and each partition writes contiguous 8 KB+ HBM runs.
"""

import ml_dtypes
import numpy as np

B, N, D = 4, 4096, 256
NCORES = 8
HALF = N // 2          # 2048 rows written per core
P = 128                # SBUF partitions
RPP = HALF // P        # 16 rows per partition (per half)
CH = 8                 # rows-per-partition per h-load/dot chunk (512 KB)
NG = RPP               # 16 output groups of [P, N] each (1 MB bf16)

_CACHE = {}


def _build():
    import concourse.mybir as mybir
    import concourse.tile as tile
    from concourse import bacc

    f32 = mybir.dt.float32
    bf16 = mybir.dt.bfloat16
    Copy = mybir.ActivationFunctionType.Copy
    Exp = mybir.ActivationFunctionType.Exp
    AX = mybir.AxisListType.X
    ADD = mybir.AluOpType.add
    MUL = mybir.AluOpType.mult
    nc = bacc.Bacc("TRN2", target_bir_lowering=False, debug=False)

    h_ext = nc.declare_dram_parameter("h", [N, D], bf16, isOutput=False)
    # w arrives pre-broadcast to [P, D] (tiny, lands first); it is repeated
    # to [P, CH, D] on DVE during the first h chunk's load so the multiplies
    # read real-strided bf16 (keeps DVE 2x mode).
    w_ext = nc.declare_dram_parameter("wb", [P, D], bf16, isOutput=False)
    out_ext = nc.declare_dram_parameter("out", [HALF, N], bf16, isOutput=True)

    # contiguous flat views: partition p holds rows 16p..16p+15 of each half
    h_oth = h_ext[0:HALF, :].rearrange("(p r) d -> p r d", p=P)
    h_mine = h_ext[HALF:N, :].rearrange("(p r) d -> p r d", p=P)
    # (q r) view of out: device row q*16 + r <-> e[q, r] (input row 16q + r),
    # so the shard comes back in natural order and partition q's writes are
    # contiguous in HBM.
    out_q = out_ext[:, :].rearrange("(q r) j -> q r j", r=RPP)

    with tile.TileContext(nc) as tc:
        with (
            tc.tile_pool(name="const", bufs=1) as cpool,
            tc.tile_pool(name="hload", bufs=4) as hpool,
            tc.tile_pool(name="prod", bufs=4) as ppool,
            tc.tile_pool(name="small", bufs=1) as spool,
            tc.tile_pool(name="obuf", bufs=5) as opool,
            tc.tile_pool(name="psum", bufs=1, space="PSUM") as psum_pool,
        ):
            # all-ones [128,128] for the PE cross-partition-sum trick
            ones_k = cpool.tile([P, P], f32)
            nc.vector.memset(ones_k[:, :], 1.0)

            # --- w (tiny, first on the sync ring), repeated on DVE while the
            # first h chunk streams in ---
            w_bc = cpool.tile([P, D], bf16)
            nc.sync.dma_start(out=w_bc[:, :], in_=w_ext[:, :])
            w_rep = cpool.tile([P, CH, D], bf16)
            nc.vector.tensor_copy(
                w_rep[:, :, :],
                w_bc[:, :].unsqueeze(1).broadcast_to([P, CH, D]),
            )

            # --- s = h @ w, other half first (its rows only feed the softmax
            # sum), own half second ending in a tiny 2-row chunk (shortens
            # the critical tail to the softmax sum). All h DMAs on the sync
            # ring, queued up-front (bufs cover all chunks). Per chunk: DVE
            # does the elementwise multiply; a few leading rows reduce on ACT
            # (accum-reduce), the rest in one batched DVE tensor_reduce. ---
            s_oth = spool.tile([P, RPP], f32)
            s_mine = spool.tile([P, RPP], f32)
            e_oth = spool.tile([P, RPP], f32)
            e_mine = spool.tile([P, RPP], f32)
            rs = spool.tile([P, 3], f32)
            jnk = spool.tile([P, D], f32)

            # (src, row0, nrows, act_rows, exp: (e_dst, e_lo, e_hi, rs_col))
            chunks = [
                (h_oth, 0, 8, 3, None),
                (h_oth, 8, 8, 3, (e_oth, 0, 16, 0)),
                (h_mine, 0, 8, 2, None),
                (h_mine, 8, 6, 2, (e_mine, 0, 14, 1)),
                (h_mine, 14, 2, 1, (e_mine, 14, 16, 2)),
            ]
            for h_src, r0, nr, act_rows, expi in chunks:
                s_dst = s_oth if h_src is h_oth else s_mine
                hch = hpool.tile([P, CH, D], bf16, tag="hch")
                nc.sync.dma_start(
                    out=hch[:, 0:nr, :], in_=h_src[:, r0 : r0 + nr, :]
                )
                prod = ppool.tile([P, CH, D], bf16, tag="prod")
                nc.vector.tensor_tensor(
                    out=prod[:, 0:nr, :],
                    in0=hch[:, 0:nr, :],
                    in1=w_rep[:, 0:nr, :],
                    op=MUL,
                )
                for g in range(act_rows):
                    nc.scalar.activation(
                        out=jnk[:, :],
                        in_=prod[:, g, :],
                        func=Copy,
                        accum_out=s_dst[:, r0 + g : r0 + g + 1],
                    )
                nc.vector.tensor_reduce(
                    out=s_dst[:, r0 + act_rows : r0 + nr],
                    in_=prod[:, act_rows:nr, :],
                    axis=AX,
                    op=ADD,
                )
                if expi is not None:
                    e_dst, lo, hi, col = expi
                    nc.scalar.activation(
                        out=e_dst[:, lo:hi],
                        in_=s_dst[:, lo:hi],
                        func=Exp,
                        accum_out=rs[:, col : col + 1],
                    )

            # --- total sum via PE ones-matmul (sums partitions, broadcasts
            # the result to every partition), then 1/S and p = e/S ---
            rs2 = spool.tile([P, 3], f32)
            nc.vector.tensor_copy(rs2[:, :], rs[:, :])  # single producer for PE
            tot_psum = psum_pool.tile([P, 3], f32)
            nc.tensor.matmul(
                tot_psum[:, 0:3], ones_k[:, 0:P], rs2[:, 0:3], start=True, stop=True
            )
            tot = spool.tile([P, 1], f32)
            nc.vector.tensor_reduce(out=tot[:, 0:1], in_=tot_psum[:, 0:3], axis=AX, op=ADD)
            inv = spool.tile([P, 1], f32)
            nc.vector.reciprocal(inv[:, 0:1], tot[:, 0:1])
            p_mine = spool.tile([P, RPP], f32)
            nc.vector.tensor_scalar_mul(p_mine[:, :], e_mine[:, :], inv[:, 0:1])

            # --- broadcast p along columns (stride-0 reads) into bf16 tiles
            # and stream out on both HWDGE rings. Schedule: two half-group
            # DMAs first (earliest first byte), two mid-stream groups on ACT
            # (parallel feed), 2-group supertiles on DVE. GpSimd is useless
            # here (measured ~7x slower than DVE and it stalls concurrent
            # DVE casts). ---
            def bcast(eng, dst, src_col):
                if eng == "act":
                    nc.scalar.activation(out=dst, in_=src_col, func=Copy)
                else:
                    nc.vector.tensor_copy(dst, src_col)

            # (n_groups, engine) per supertile; groups assigned sequentially
            sched = [
                (1, "dve"), (1, "act"), (2, "dve"), (1, "act"), (2, "dve"),
                (2, "dve"), (2, "dve"), (2, "dve"), (2, "dve"), (1, "dve"),
            ]
            gi = 0
            for t, (width, eng) in enumerate(sched):
                ot = opool.tile([P, 2 * N], bf16, tag="ot")
                if t == 0:
                    # first group split into two half-row DMAs for fast start
                    for hj in range(2):
                        j0 = hj * (N // 2)
                        bcast(
                            eng,
                            ot[:, j0 : j0 + N // 2],
                            p_mine[:, 0:1].broadcast_to([P, N // 2]),
                        )
                        dma_eng = nc.sync if hj == 0 else nc.scalar
                        dma_eng.dma_start(
                            out=out_q[:, 0:1, j0 : j0 + N // 2],
                            in_=ot[:, j0 : j0 + N // 2].rearrange(
                                "q (r j) -> q r j", r=1
                            ),
                        )
                    gi += 1
                    continue
                if width == 1:
                    bcast(eng, ot[:, 0:N], p_mine[:, gi : gi + 1].broadcast_to([P, N]))
                else:
                    bcast(
                        eng,
                        ot[:, 0 : 2 * N].rearrange("q (r j) -> q r j", r=2),
                        p_mine[:, gi : gi + 2].unsqueeze(2).broadcast_to([P, 2, N]),
                    )
                dma_eng = nc.sync if t % 2 == 0 else nc.scalar
                dma_eng.dma_start(
                    out=out_q[:, gi : gi + width, :],
                    in_=ot[:, 0 : width * N].rearrange("q (r j) -> q r j", r=width),
                )
                gi += width
            assert gi == NG
    nc.compile()
    return nc


def _get_nc():
    if "nc" not in _CACHE:
        _CACHE["nc"] = _build()
    return _CACHE["nc"]


def _ensure_axon_hooks():
    """bass_utils' trace path imports antenv.axon_hooks, which some images
    lack; provide a stub so tracing degrades instead of crashing. If the
    boot package + libaxon_pjrt.so are present, register the real
    ctypes-based NTFF profile hook so traced runs report exec_time_ns."""
    import sys
    import types

    try:
        import antenv.axon_hooks as m
    except ImportError:
        try:
            import antenv
        except ImportError:
            antenv = types.ModuleType("antenv")
            sys.modules["antenv"] = antenv
        m = types.ModuleType("antenv.axon_hooks")
        m._hook = None
        m.set_axon_ntff_profile_hook = lambda h: setattr(m, "_hook", h)
        m.get_axon_ntff_profile_hook = lambda: m._hook
        sys.modules["antenv.axon_hooks"] = m
    if m.get_axon_ntff_profile_hook() is None:
        try:
            import os

            from trn_agent_boot.trn_boot import _ntff_profile_via_ctypes

            so_path = "/opt/axon/libaxon_pjrt.so"
            if os.path.exists(so_path):
                hook = _ntff_profile_via_ctypes(so_path)
                if hook is not None:
                    m.set_axon_ntff_profile_hook(hook)
        except Exception:
            pass


def run_on_device(h, w, trace=False):
    """Run the SPMD kernel; returns the BassKernelResults."""
    from concourse.bass_utils import run_bass_kernel_spmd

    _ensure_axon_hooks()

    wb = np.ascontiguousarray(
        np.broadcast_to(w.astype(ml_dtypes.bfloat16), (P, D))
    )
    in_maps = []
    for c in range(NCORES):
        b_idx, rh = divmod(c, 2)
        hb = h[b_idx]
        # other half first (off the critical path), own half second
        if rh:
            hb_dev = hb
        else:
            hb_dev = np.concatenate([hb[HALF:], hb[:HALF]], axis=0)
        in_maps.append(
            {
                "h": np.ascontiguousarray(hb_dev.astype(ml_dtypes.bfloat16)),
                "wb": wb,
            }
        )
    res = run_bass_kernel_spmd(
        _get_nc(), in_maps, core_ids=list(range(NCORES)), trace=trace
    )
    return res


def kernel(h, w, b):
    h = np.asarray(h, dtype=np.float32)
    w = np.asarray(w, dtype=np.float32)
    res = run_on_device(h, w)
    A = np.empty((B, N, N), dtype=np.float32)
    for c in range(NCORES):
        b_idx, rh = divmod(c, 2)
        A[b_idx, rh * HALF : (rh + 1) * HALF, :] = res.results[c]["out"].astype(
            np.float32
        )
    return A


# revision 11
# speedup vs baseline: 1.5206x; 1.0823x over previous
"""Trainium2 Bass kernel for nn_AdaptiveAdjacencyMatrix.

Reference math:
    s[b, i]        = sum_d h[b, i, d] * w[d]
    scores[b,i,j]  = s[b,i] + s[b,j] + bias
    A              = softmax(scores, axis=1)   # over i

Because the softmax is over axis=1 (i), the `s[b,j] + bias` term is constant
along the reduced axis and cancels exactly:
    A[b, i, j] = exp(s[b,i]) / sum_i' exp(s[b,i'])   (independent of j and bias)

So the output is a column-broadcast of softmax(s[b]) — the kernel is purely
memory-bound. The output is written in bf16 (the host upcasts to f32), which
halves HBM write traffic vs f32; quantization error ~2^-9 is far inside the
accuracy budget.

Sharding: 8 cores = (batch b, row-half rh). Each core receives the full
h[b] (rows reordered so its own 2048 rows come LAST; the other half comes
first so its dot products are off the critical path), computes the full
softmax sum locally (needs all 4096 rows; row order is irrelevant to the
sum), and writes a [2048, 4096] bf16 output shard. No collectives.

Layouts: h is DMA'd with contiguous per-partition descriptors (partition p
holds rows 16p..16p+15 of a half, 4 KB runs per chunk). The output uses the
matching (q r) layout — device row q*16 + r holds the value for input row
16q + r — so the returned shard is already in natural row order (no host
permute) and each partition writes contiguous 8 KB+ HBM runs.
"""

import ml_dtypes
import numpy as np

B, N, D = 4, 4096, 256
NCORES = 8
HALF = N // 2          # 2048 rows written per core
P = 128                # SBUF partitions
RPP = HALF // P        # 16 rows per partition (per half)
CH = 8                 # rows-per-partition per h-load/dot chunk (512 KB)
NG = RPP               # 16 output groups of [P, N] each (1 MB bf16)

_CACHE = {}


def _build():
    import concourse.mybir as mybir
    import concourse.tile as tile
    from concourse import bacc

    f32 = mybir.dt.float32
    bf16 = mybir.dt.bfloat16
    Copy = mybir.ActivationFunctionType.Copy
    Exp = mybir.ActivationFunctionType.Exp
    AX = mybir.AxisListType.X
    ADD = mybir.AluOpType.add
    MUL = mybir.AluOpType.mult
    nc = bacc.Bacc("TRN2", target_bir_lowering=False, debug=False)

    h_ext = nc.declare_dram_parameter("h", [N, D], bf16, isOutput=False)
    # w arrives pre-broadcast to [P, D] (tiny, lands first); it is repeated
    # to [P, CH, D] on DVE during the first h chunk's load so the multiplies
    # read real-strided bf16 (keeps DVE 2x mode).
    w_ext = nc.declare_dram_parameter("wb", [P, D], bf16, isOutput=False)
    out_ext = nc.declare_dram_parameter("out", [HALF, N], bf16, isOutput=True)

    # contiguous flat views: partition p holds rows 16p..16p+15 of each half
    h_oth = h_ext[0:HALF, :].rearrange("(p r) d -> p r d", p=P)
    h_mine = h_ext[HALF:N, :].rearrange("(p r) d -> p r d", p=P)
    # (q r) view of out: device row q*16 + r <-> e[q, r] (input row 16q + r),
    # so the shard comes back in natural order and partition q's writes are
    # contiguous in HBM.
    out_q = out_ext[:, :].rearrange("(q r) j -> q r j", r=RPP)

    with tile.TileContext(nc) as tc:
        with (
            tc.tile_pool(name="const", bufs=1) as cpool,
            tc.tile_pool(name="hload", bufs=5) as hpool,
            tc.tile_pool(name="prod", bufs=4) as ppool,
            tc.tile_pool(name="small", bufs=1) as spool,
            tc.tile_pool(name="obuf", bufs=8) as opool,
            tc.tile_pool(name="psum", bufs=1, space="PSUM") as psum_pool,
        ):
            # all-ones [128,128] for the PE cross-partition-sum trick
            ones_k = cpool.tile([P, P], f32)
            nc.vector.memset(ones_k[:, :], 1.0)

            # --- w (tiny, first on the sync ring), repeated on DVE while the
            # first h chunk streams in ---
            w_bc = cpool.tile([P, D], bf16)
            nc.sync.dma_start(out=w_bc[:, :], in_=w_ext[:, :])
            w_rep = cpool.tile([P, CH, D], bf16)
            nc.vector.tensor_copy(
                w_rep[:, :, :],
                w_bc[:, :].unsqueeze(1).broadcast_to([P, CH, D]),
            )

            # --- s = h @ w, other half first (its rows only feed the softmax
            # sum), own half second ending in a tiny 2-row chunk (shortens
            # the critical tail to the softmax sum). All h DMAs on the sync
            # ring, queued up-front (bufs cover all chunks). Per chunk: DVE
            # does the elementwise multiply; a few leading rows reduce on ACT
            # (accum-reduce), the rest in one batched DVE tensor_reduce. ---
            s_oth = spool.tile([P, RPP], f32)
            s_mine = spool.tile([P, RPP], f32)
            e_oth = spool.tile([P, RPP], f32)
            e_mine = spool.tile([P, RPP], f32)
            rs = spool.tile([P, 3], f32)
            jnk = spool.tile([P, D], f32)

            # (src, row0, nrows, act_rows, exp: (e_dst, e_lo, e_hi, rs_col))
            chunks = [
                (h_oth, 0, 8, 3, None),
                (h_oth, 8, 8, 2, (e_oth, 0, 16, 0)),
                (h_mine, 0, 8, 2, None),
                (h_mine, 8, 6, 2, (e_mine, 0, 14, 1)),
                (h_mine, 14, 2, 1, (e_mine, 14, 16, 2)),
            ]
            for ci, (h_src, r0, nr, act_rows, expi) in enumerate(chunks):
                s_dst = s_oth if h_src is h_oth else s_mine
                hch = hpool.tile([P, CH, D], bf16, tag="hch")
                h_dma_eng = nc.sync if ci % 2 == 0 else nc.scalar
                h_dma_eng.dma_start(
                    out=hch[:, 0:nr, :], in_=h_src[:, r0 : r0 + nr, :]
                )
                prod = ppool.tile([P, CH, D], bf16, tag="prod")
                nc.vector.tensor_tensor(
                    out=prod[:, 0:nr, :],
                    in0=hch[:, 0:nr, :],
                    in1=w_rep[:, 0:nr, :],
                    op=MUL,
                )
                for g in range(act_rows):
                    nc.scalar.activation(
                        out=jnk[:, :],
                        in_=prod[:, g, :],
                        func=Copy,
                        accum_out=s_dst[:, r0 + g : r0 + g + 1],
                    )
                nc.vector.tensor_reduce(
                    out=s_dst[:, r0 + act_rows : r0 + nr],
                    in_=prod[:, act_rows:nr, :],
                    axis=AX,
                    op=ADD,
                )
                if expi is not None:
                    e_dst, lo, hi, col = expi
                    nc.scalar.activation(
                        out=e_dst[:, lo:hi],
                        in_=s_dst[:, lo:hi],
                        func=Exp,
                        accum_out=rs[:, col : col + 1],
                    )

            # --- total sum via PE ones-matmul (sums partitions, broadcasts
            # the result to every partition), then 1/S and p = e/S ---
            rs2 = spool.tile([P, 3], f32)
            nc.vector.tensor_copy(rs2[:, :], rs[:, :])  # single producer for PE
            tot_psum = psum_pool.tile([P, 3], f32)
            nc.tensor.matmul(
                tot_psum[:, 0:3], ones_k[:, 0:P], rs2[:, 0:3], start=True, stop=True
            )
            tot = spool.tile([P, 1], f32)
            nc.vector.tensor_reduce(out=tot[:, 0:1], in_=tot_psum[:, 0:3], axis=AX, op=ADD)
            inv = spool.tile([P, 1], f32)
            nc.vector.reciprocal(inv[:, 0:1], tot[:, 0:1])
            p_mine = spool.tile([P, RPP], f32)
            nc.vector.tensor_scalar_mul(p_mine[:, :], e_mine[:, :], inv[:, 0:1])

            # --- broadcast p along columns (stride-0 reads) into bf16 tiles
            # and stream out on both HWDGE rings. Schedule: two half-group
            # DMAs first (earliest first byte), two mid-stream groups on ACT
            # (parallel feed), 2-group supertiles on DVE. GpSimd is useless
            # here (measured ~7x slower than DVE and it stalls concurrent
            # DVE casts). ---
            def bcast(eng, dst, src_col):
                if eng == "act":
                    nc.scalar.activation(out=dst, in_=src_col, func=Copy)
                else:
                    nc.vector.tensor_copy(dst, src_col)

            # One DMA per output group (fine interleave keeps both queues
            # evenly fed to the end); group 0 split into two half-row DMAs
            # for the earliest first byte. Engines: two mid-stream groups on
            # ACT (parallel feed while DVE casts), the rest on DVE.
            ACT_GROUPS = (1, 4)
            nd = 0  # dma counter for queue alternation
            for g in range(NG):
                eng = "act" if g in ACT_GROUPS else "dve"
                if g == 0:
                    ot = opool.tile([P, N], bf16, tag="ot")
                    for hj in range(2):
                        j0 = hj * (N // 2)
                        bcast(
                            eng,
                            ot[:, j0 : j0 + N // 2],
                            p_mine[:, 0:1].broadcast_to([P, N // 2]),
                        )
                        dma_eng = nc.sync if nd % 2 == 0 else nc.scalar
                        nd += 1
                        dma_eng.dma_start(
                            out=out_q[:, 0:1, j0 : j0 + N // 2],
                            in_=ot[:, j0 : j0 + N // 2].rearrange(
                                "q (r j) -> q r j", r=1
                            ),
                        )
                    continue
                ot = opool.tile([P, N], bf16, tag="ot")
                bcast(eng, ot[:, 0:N], p_mine[:, g : g + 1].broadcast_to([P, N]))
                dma_eng = nc.sync if nd % 2 == 0 else nc.scalar
                nd += 1
                dma_eng.dma_start(
                    out=out_q[:, g : g + 1, :],
                    in_=ot[:, 0:N].rearrange("q (r j) -> q r j", r=1),
                )
    nc.compile()
    return nc


def _get_nc():
    if "nc" not in _CACHE:
        _CACHE["nc"] = _build()
    return _CACHE["nc"]


def _ensure_axon_hooks():
    """bass_utils' trace path imports antenv.axon_hooks, which some images
    lack; provide a stub so tracing degrades instead of crashing. If the
    boot package + libaxon_pjrt.so are present, register the real
    ctypes-based NTFF profile hook so traced runs report exec_time_ns."""
    import sys
    import types

    try:
        import antenv.axon_hooks as m
    except ImportError:
        try:
            import antenv
        except ImportError:
            antenv = types.ModuleType("antenv")
            sys.modules["antenv"] = antenv
        m = types.ModuleType("antenv.axon_hooks")
        m._hook = None
        m.set_axon_ntff_profile_hook = lambda h: setattr(m, "_hook", h)
        m.get_axon_ntff_profile_hook = lambda: m._hook
        sys.modules["antenv.axon_hooks"] = m
    if m.get_axon_ntff_profile_hook() is None:
        try:
            import os

            from trn_agent_boot.trn_boot import _ntff_profile_via_ctypes

            so_path = "/opt/axon/libaxon_pjrt.so"
            if os.path.exists(so_path):
                hook = _ntff_profile_via_ctypes(so_path)
                if hook is not None:
                    m.set_axon_ntff_profile_hook(hook)
        except Exception:
            pass


def run_on_device(h, w, trace=False):
    """Run the SPMD kernel; returns the BassKernelResults."""
    from concourse.bass_utils import run_bass_kernel_spmd

    _ensure_axon_hooks()

    wb = np.ascontiguousarray(
        np.broadcast_to(w.astype(ml_dtypes.bfloat16), (P, D))
    )
    in_maps = []
    for c in range(NCORES):
        b_idx, rh = divmod(c, 2)
        hb = h[b_idx]
        # other half first (off the critical path), own half second
        if rh:
            hb_dev = hb
        else:
            hb_dev = np.concatenate([hb[HALF:], hb[:HALF]], axis=0)
        in_maps.append(
            {
                "h": np.ascontiguousarray(hb_dev.astype(ml_dtypes.bfloat16)),
                "wb": wb,
            }
        )
    res = run_bass_kernel_spmd(
        _get_nc(), in_maps, core_ids=list(range(NCORES)), trace=trace
    )
    return res


def kernel(h, w, b):
    h = np.asarray(h, dtype=np.float32)
    w = np.asarray(w, dtype=np.float32)
    res = run_on_device(h, w)
    A = np.empty((B, N, N), dtype=np.float32)
    for c in range(NCORES):
        b_idx, rh = divmod(c, 2)
        A[b_idx, rh * HALF : (rh + 1) * HALF, :] = res.results[c]["out"].astype(
            np.float32
        )
    return A
